# revision 11
# baseline (speedup 1.0000x reference)
"""Trainium2 Bass kernel for nn_MultiHeadNetwork (moe_routing).

Strategy
--------
Host side (numpy, inside kernel()):
  * task id per row = argmax of the trailing one-hot block of x (data, not
    activation dependent).  Rows are grouped by task and cut into "pieces":
    floor(r/64) pieces of exactly 64 rows per task plus one remainder piece
    (<64 rows).  Pieces are assigned to cores (512 rows each, 64-pieces as
    filler, remainder pieces grouped by subset-sum DP so each core's
    remainders sum to a multiple of 64).  Every piece becomes one "window"
    of the head phase; the shared per-core window count N is the SPMD
    program shape (13 here = ceil(98/8), the structural optimum).
  * Trunk weights replicated across cores.  Head weights are packed per
    window (a core's window w gets its piece's task head weights).
  * Everything is converted to bfloat16 on host (halves DMA + H2D vs fp32;
    the PE runs bf16 at the same 1 col/cycle rate as fp32r, and bf16 enables
    fast weight loads).
  * The one-hot block of x contributes W0[2048+tid] + b0 per row, so layer 0
    runs over the 2048 dense features only (16 k-chunks instead of 17); the
    per-row contribution is host-gathered and added on the vector engine.
  * Layer-0 inputs (xT and the one-hot contribution) are packed as fused
    chunk-pairs [8, 128, 1024] so their DMAs read 2 KiB per partition line.

Device side (one SPMD Tile program on 8 cores):
  * Activations feature-major (hT: [feat partitions, batch free]) so each
    trunk layer is out = W_chunk.T @ hT with NO transposes anywhere.
  * Trunk layers run k-OUTER over 8-chunk groups (8 PSUM banks): weight DMAs
    are [128, 1024] bf16 (2 KiB / partition line), prefetched deep; ReLU +
    bias fused on the scalar engine straight out of PSUM (layer 0: DVE add of
    the one-hot contribution + ReLU), output bf16.
  * Head (flipped operands): per window w the head weights are STATIONARY
    ([128 k, 128 h] chunks) and the moving operand is a 64-column slice of
    h3 at a per-core register offset (loaded from a tiny offsets table via
    values_load -- the window position is data, the program is SPMD
    uniform).  Matmul cost scales with the 64-column window instead of the
    256-wide head dim: 13 windows x 2 halves x 16 k x 64 cycles ~= 26.6k
    cycles vs 40.9k for the mask-merged slot scheme.  A predicated DVE copy
    (per-core 0/1 column masks) stitches each window's [128, 64] PSUM block
    into the transposed output tiles [2][128 h, 512 rows].
  * Head weight DMAs are dripped during L1/L2 so they complete during the
    trunk.  Head bias and the inverse permutation are applied on host.
"""

import numpy as np
import ml_dtypes
from contextlib import ExitStack

import concourse.bacc as bacc
import concourse.mybir as mybir
from concourse.tile import TileContext
from concourse.bass import ds
from concourse import bass_utils

BATCH = 4096
FEAT = 2048
NUM_TASKS = 50
WIDTH = 2048
HEAD_DIM = 256
NCORES = 8
BPC = BATCH // NCORES          # 512 rows per core
KC = WIDTH // 128              # 16 contraction chunks per layer
WC = WIDTH // 128              # 16 w-chunks per layer
GW = 8                         # w-chunks per trunk group (8 PSUM banks)
NG = WC // GW                  # 2 groups per layer
WIN = 64                       # head window width (= bf16 FWL LDWEIGHTS II)

F32 = mybir.dt.float32
BF16 = mybir.dt.bfloat16
U8 = mybir.dt.uint8
I32 = mybir.dt.int32
NPBF16 = ml_dtypes.bfloat16

_PROG_CACHE: dict = {}
_PACK_CACHE: dict = {}


def _fingerprint(*arrs):
    """Cheap identity+content fingerprint for caching packed weights."""
    parts = []
    for a in arrs:
        s = np.asarray(a).reshape(-1)
        step = max(1, s.size // 64)
        parts.append((id(a), s.size, float(s[::step][:64].astype(np.float64).sum())))
    return tuple(parts)


def _build(nwin: int, repeat: int = 1, bench: bool = False):
    """Build + compile the SPMD Tile program.

    nwin = shared per-core head window count (cores with fewer real windows
    get zero-masked padding windows).  repeat > 1 wraps the body in a
    hardware For_i loop (benchmarking only).  bench=True turns the big
    inputs into Internal (device-resident scratch) tensors so a benchmark
    run has no big H2D transfers.
    """
    nwin = int(nwin)
    nc = bacc.Bacc("TRN2", target_bir_lowering=False, debug=False)
    kind = "Internal" if bench else "ExternalInput"
    xT = nc.dram_tensor("xT", [KC // 2, 128, 2 * BPC], BF16, kind=kind).ap()
    ctb = nc.dram_tensor("ctb", [KC // 2, 128, 2 * BPC], BF16, kind=kind).ap()
    w0 = nc.dram_tensor("w0p", [NG, KC, 128, GW * 128], BF16, kind=kind).ap()
    w1 = nc.dram_tensor("w1p", [NG, KC, 128, GW * 128], BF16, kind=kind).ap()
    w2 = nc.dram_tensor("w2p", [NG, KC, 128, GW * 128], BF16, kind=kind).ap()
    bia = nc.dram_tensor("bias", [128, 3 * WC], F32, kind=kind).ap()
    hws = nc.dram_tensor("hws", [nwin, 128, KC * HEAD_DIM], BF16, kind=kind).ap()
    msk = nc.dram_tensor("msk", [128, nwin * WIN], U8, kind=kind).ap()
    offs = nc.dram_tensor("offs", [1, nwin], I32, kind=kind).ap()
    if bench:
        dummy = nc.dram_tensor("bmark_in", [128, 16], F32, kind="ExternalInput").ap()
        outk = "Internal"
    else:
        outk = "ExternalOutput"
    out = nc.dram_tensor("outT", [2, 128, BPC], F32, kind=outk).ap()
    if bench:
        outb = nc.dram_tensor("outb", [128, 16], F32, kind="ExternalOutput").ap()

    with TileContext(nc) as tc, ExitStack() as ctx:
        # xT (16 tiles) and h2 (16) share slots: h2 allocates only after
        # layer 0 fully finished reading xT.  h1/h3 share the other pool.
        actA = ctx.enter_context(tc.tile_pool(name="actA", bufs=KC))
        xtp = ctx.enter_context(tc.tile_pool(name="xtp", bufs=KC // 2))
        actB = ctx.enter_context(tc.tile_pool(name="actB", bufs=KC))
        wp = ctx.enter_context(tc.tile_pool(name="wp", bufs=13))
        cons = ctx.enter_context(tc.tile_pool(name="cons", bufs=1))
        ctp = ctx.enter_context(tc.tile_pool(name="ctp", bufs=8))
        hwp = ctx.enter_context(tc.tile_pool(name="hwp", bufs=nwin))
        op = ctx.enter_context(tc.tile_pool(name="op", bufs=4))
        psp = ctx.enter_context(tc.tile_pool(name="psp", bufs=8, space="PSUM"))

        if bench:
            # one-time (outside the loop) init of the Internal scratch
            # tensors: uninitialized HBM can hold NaN/denormal bit patterns
            # that would skew the timing vs real data
            with tc.tile_pool(name="initp", bufs=1) as initp:
                zt = initp.tile([128, GW * 128], BF16, tag="zt")
                nc.vector.memset(zt[:], 0.125)
                for g in range(NG):
                    for k in range(KC):
                        nc.sync.dma_start(w0[g, k], zt[:])
                        nc.sync.dma_start(w1[g, k], zt[:])
                        nc.sync.dma_start(w2[g, k], zt[:])
                for k in range(KC // 2):
                    nc.sync.dma_start(xT[k], zt[:])
                    nc.sync.dma_start(ctb[k], zt[:])
                for w in range(nwin):
                    for k in range(KC):
                        nc.sync.dma_start(
                            hws[w][:, k * HEAD_DIM:(k + 1) * HEAD_DIM],
                            zt[:, :HEAD_DIM])
                ztf = initp.tile([128, 3 * WC], F32, tag="ztf")
                nc.vector.memset(ztf[:], 0.0)
                nc.sync.dma_start(bia, ztf[:])
                ztm = initp.tile([128, nwin * WIN], U8, tag="ztm")
                nc.vector.memset(ztm[:], 1)
                nc.sync.dma_start(msk, ztm[:])
                zto = initp.tile([1, nwin], I32, tag="zto")
                nc.vector.memset(zto[:], 0)
                nc.sync.dma_start(offs, zto[:])

        if repeat > 1:
            ctx.enter_context(tc.For_i(0, repeat, 1))

        xt = [None] * KC

        # head window offsets: tiny DMA first, then preload into PE + DVE
        # registers during the initial weight-DMA wait
        ot = cons.tile([1, nwin], I32, tag="ot")
        nc.sync.dma_start(ot[:], offs)
        offv = []
        for w in range(nwin):
            offv.append(nc.values_load(
                ot[0:1, w:w + 1],
                engines=[mybir.EngineType.PE, mybir.EngineType.DVE],
                min_val=0, max_val=BPC - WIN,
                skip_runtime_bounds_check=True,
            ))

        # head-weight prefetch queue: dripped one DMA per few trunk
        # k-iterations (a single burst would starve the trunk weight DMAs)
        hw = [hwp.tile([128, KC * HEAD_DIM], BF16, tag="hwp", name=f"hw{w}")
              for w in range(nwin)]
        mt = cons.tile([128, nwin * WIN], U8, tag="mt")
        bt = cons.tile([128, 3 * WC], F32, tag="bt")
        drip_q = [(mt, msk)] + [(hw[w], hws[w]) for w in range(nwin)]

        ctt = [None] * KC

        def trunk_layer(src, wdram, nk, li, pool, tag, load_x=False, drip=0):
            outs = [None] * WC
            it = 0
            for g in range(NG):
                pss = [
                    psp.tile([128, BPC], F32, tag="ps", name=f"psL{li}g{g}w{w}")
                    for w in range(GW)
                ]
                for k in range(nk):
                    wt = wp.tile([128, GW * 128], BF16, tag="wp", name=f"wtL{li}g{g}k{k}")
                    if li == 0 and g == 0 and k == 0:
                        # split the very first weight DMA so the first
                        # stationary [128,128] chunk lands as early as possible
                        nc.sync.dma_start(wt[:, :128], wdram[g, k][:, :128])
                        nc.sync.dma_start(wt[:, 128:], wdram[g, k][:, 128:])
                    else:
                        nc.sync.dma_start(wt[:], wdram[g, k])
                    if load_x and g == 0 and k % 2 == 0:
                        t = xtp.tile([128, 2 * BPC], BF16, tag="xtp",
                                     name=f"xt{k}")
                        nc.sync.dma_start(t[:], xT[k // 2])
                        src[k] = t[:, :BPC]
                        src[k + 1] = t[:, BPC:]
                        if k == 0:
                            # bias: small DMA, must be traced before the
                            # first ReLU that reads it
                            nc.sync.dma_start(bt[:], bia)
                    if load_x and k % 4 == 1:
                        # fused pair of one-hot contribution chunks for this
                        # group: 2 KiB partition lines instead of 1 KiB
                        pair = g * (GW // 2) + k // 4
                        ct2 = ctp.tile([128, 2 * BPC], BF16, tag="ctp",
                                       name=f"ct2_{pair}")
                        nc.sync.dma_start(ct2[:], ctb[pair])
                        ctt[2 * pair] = ct2[:, :BPC]
                        ctt[2 * pair + 1] = ct2[:, BPC:]
                    if drip and drip_q and it % drip == drip - 1:
                        tile, src_ap = drip_q.pop(0)
                        nc.sync.dma_start(tile[:], src_ap)
                    it += 1
                    for w in range(GW):
                        nc.tensor.matmul(
                            pss[w][:],
                            wt[:, w * 128:(w + 1) * 128],
                            src[k] if load_x else src[k][:],
                            start=(k == 0),
                            stop=(k == nk - 1),
                        )
                for w in range(GW):
                    wc_i = g * GW + w
                    h = pool.tile([128, BPC], BF16, tag=tag, name=f"h{li}_{wc_i}")
                    if li == 0:
                        # layer 0: the one-hot block of x contributes
                        # W0[2048+tid] + b0 per row (host-gathered): add on
                        # DVE straight out of PSUM, then ReLU in place
                        nc.vector.tensor_tensor(
                            h[:], pss[w][:], ctt[wc_i], mybir.AluOpType.add)
                        nc.vector.tensor_relu(h[:], h[:])
                    else:
                        nc.scalar.activation(
                            h[:], pss[w][:], mybir.ActivationFunctionType.Relu,
                            bias=bt[:, li * WC + wc_i: li * WC + wc_i + 1],
                        )
                    outs[wc_i] = h
            return outs

        h1 = trunk_layer(xt, w0, KC, 0, actB, "actB", load_x=True)
        # lighter drip during L1 (its wt DMAs + drips are the heaviest HBM
        # stretch under 8-core contention), denser during L2
        h2 = trunk_layer(h1, w1, KC, 1, actA, "actA", drip=3)
        h3 = trunk_layer(h2, w2, KC, 2, actB, "actB", drip=2)
        for tile, src_ap in drip_q:
            nc.sync.dma_start(tile[:], src_ap)
        drip_q.clear()

        # head: window w covers 64 h3 columns at per-core register offset;
        # head weights stationary [128 k, 128 h-half]; predicated DVE copy
        # stitches the window's PSUM block into the transposed output
        ob = [op.tile([128, BPC], F32, tag="op", name=f"ob{half}")
              for half in range(2)]
        for w in range(nwin):
            ps = [psp.tile([128, WIN], F32, tag="ps", name=f"psH{w}_{half}")
                  for half in range(2)]
            for k in range(KC):
                mov = h3[k][:, ds(offv[w], WIN)]
                for half in range(2):
                    nc.tensor.matmul(
                        ps[half][:],
                        hw[w][:, k * HEAD_DIM + half * 128:
                              k * HEAD_DIM + half * 128 + 128],
                        mov,
                        start=(k == 0),
                        stop=(k == KC - 1),
                    )
            for half in range(2):
                nc.vector.copy_predicated(
                    ob[half][:, ds(offv[w], WIN)],
                    mt[:, w * WIN:(w + 1) * WIN],
                    ps[half][:],
                )
        for half in range(2):
            nc.sync.dma_start(out[half], ob[half][:])

        if bench:
            dt = cons.tile([128, 16], F32, tag="dt")
            nc.sync.dma_start(dt[:], dummy)
            nc.vector.tensor_copy(dt[:], ob[1][:, :16])
            nc.sync.dma_start(outb, dt[:])

    nc.compile()
    return nc


def _pack_w(W, nk):
    # [NG, nk, 128, GW*128]; [g, k, kp, w*128+m] = W[k*128+kp, (g*GW+w)*128+m]
    return np.ascontiguousarray(
        W.reshape(nk, 128, NG, GW * 128).transpose(2, 0, 1, 3)
    )


def _pack_trunk(W0, W1, W2, b1, b2):
    w0p = _pack_w(W0[:FEAT].astype(NPBF16), KC)
    w1p = _pack_w(W1.astype(NPBF16), KC)
    w2p = _pack_w(W2.astype(NPBF16), KC)
    bias = np.zeros((128, 3 * WC), np.float32)
    for li, b in ((1, b1), (2, b2)):
        bias[:, li * WC:(li + 1) * WC] = b.reshape(WC, 128).T
    return w0p, w1p, w2p, bias


def _pack_dp(counts):
    """Strategy A: cut tasks into 64-row pieces + one remainder (<64);
    remainder pieces grouped per core to 64-multiple sums via subset-sum
    DP, 64-pieces fill the rest.  Near-optimal when it works; raises on
    pathological tails (caller falls back to _pack_seq)."""
    full64 = []            # (task,) one entry per 64-piece
    rems = []              # (size, task)
    for t, c in enumerate(counts):
        c = int(c)
        if c == 0:
            continue
        full64.extend([t] * (c // 64))
        if c % 64:
            rems.append((c % 64, t))
    rems.sort(reverse=True)

    total_rem = sum(s for s, _ in rems)
    assert total_rem % 64 == 0  # 4096 - 64*len(full64)

    # Windows per core = 8 + p_c where p_c = n_rem,c - sum_rem,c/64 and
    # sum p_c = len(rems) - total_rem/64 is FIXED -- so balancing p across
    # cores is the whole game.  Per core: DP over remaining remainder pieces
    # for achievable (sum, count) with sum a multiple of 64, pick a subset
    # with p == ceil(remaining_p / cores_left) and sum near the even split.
    core_rems = []
    rem_p = len(rems) - total_rem // 64
    for c in range(NCORES):
        cores_left = NCORES - c
        if cores_left == 1:
            core_rems.append(list(rems))
            rems = []
            total_rem = 0
            break
        p_tgt = -(-rem_p // cores_left)  # ceil
        vol_tgt = 64 * int(round(total_rem / 64 / cores_left))
        # best[(sum, n)] = picks
        best = {(0, 0): []}
        for i, (s, t) in enumerate(rems):
            for (acc, n), picks in list(best.items()):
                key = (acc + s, n + 1)
                if key[0] <= BPC and key not in best:
                    best[key] = picks + [i]
        pick = None
        for p_try in (p_tgt, p_tgt + 1, p_tgt - 1, p_tgt + 2, p_tgt - 2):
            cands = [(abs(acc - vol_tgt), acc, n)
                     for (acc, n) in best
                     if acc % 64 == 0 and n - acc // 64 == p_try
                     and acc <= total_rem]
            if cands:
                cands.sort()
                _, acc, n = cands[0]
                pick = best[(acc, n)]
                break
        if pick is None:
            # split one remainder piece to reach a 64-multiple volume:
            # either grow the pick by (64 - acc%64) rows cut from an
            # unpicked piece, or shrink a picked piece by acc%64 rows
            done = False
            for _, acc, n in sorted((abs(acc - vol_tgt), acc, n)
                                    for (acc, n) in best
                                    if 0 < acc < total_rem and acc % 64 != 0):
                pick = list(best[(acc, n)])
                need = 64 - acc % 64
                cut = acc % 64
                left = [i for i in range(len(rems)) if i not in pick]
                grow = [i for i in left if rems[i][0] > need]
                if grow:
                    j = max(grow, key=lambda i: rems[i][0])
                    s, t = rems[j]
                    rems[j] = (s - need, t)
                    chosen = [rems[i] for i in pick] + [(need, t)]
                    vol = acc + need
                else:
                    shrink = [i for i in pick if rems[i][0] > cut]
                    if not shrink:
                        continue
                    j = max(shrink, key=lambda i: rems[i][0])
                    s, t = rems[j]
                    chosen = [rems[i] for i in pick if i != j] + [(s - cut, t)]
                    rems[j] = (cut, t)
                    pick = [i for i in pick if i != j]
                    vol = acc - cut
                rems = [r for i, r in enumerate(rems) if i not in pick]
                core_rems.append(chosen)
                total_rem -= vol
                rem_p = len(rems) - total_rem // 64
                done = True
                break
            assert done, "remainder split failed"
            continue
        chosen = [rems[i] for i in pick]
        rems = [r for i, r in enumerate(rems) if i not in pick]
        core_rems.append(chosen)
        total_rem -= sum(x for x, _ in chosen)
        rem_p = len(rems) - total_rem // 64
    assert not rems and total_rem == 0
    assert len(core_rems) == NCORES

    # fill cores with 64-pieces; group same-task 64-pieces adjacently
    full64.sort()
    core_pieces = []
    fi = 0
    for c in range(NCORES):
        rsum = sum(s for s, _ in core_rems[c])
        n64 = (BPC - rsum) // 64
        assert n64 >= 0
        pieces = [(64, t) for t in full64[fi:fi + n64]]
        fi += n64
        pieces += [(int(s), int(t)) for s, t in core_rems[c]]
        core_pieces.append(pieces)
    assert fi == len(full64)
    return core_pieces


def _pack_seq(counts):
    """Strategy B (robust): whole tasks in zigzag size order laid out
    sequentially, cut at 512-row core boundaries."""
    tasks = sorted(((int(c), t) for t, c in enumerate(counts) if c),
                   reverse=True)
    zig = []
    lo, hi = 0, len(tasks) - 1
    while lo <= hi:
        zig.append(tasks[lo])
        lo += 1
        if lo <= hi:
            zig.append(tasks[hi])
            hi -= 1
    core_pieces = [[] for _ in range(NCORES)]
    c = 0
    cap = BPC
    for sz, t in zig:
        while sz > 0:
            take = min(sz, cap)
            core_pieces[c].append((take, t))
            sz -= take
            cap -= take
            if cap == 0 and c < NCORES - 1:
                c += 1
                cap = BPC
    assert cap == 0
    return core_pieces


def _pack_lpt(counts):
    """Strategy C (robust for skew): tasks desc by size, each placed on the
    core that minimizes its window count, splitting over cores as needed."""
    tasks = sorted(((int(c), t) for t, c in enumerate(counts) if c),
                   reverse=True)
    core_pieces = [[] for _ in range(NCORES)]
    cap = [BPC] * NCORES
    wins = [0] * NCORES
    for sz, t in tasks:
        while sz > 0:
            fit = [c for c in range(NCORES) if cap[c] >= sz]
            if fit:
                c = min(fit, key=lambda c: (wins[c] + -(-sz // WIN), -cap[c]))
                take = sz
            else:
                c = max(range(NCORES), key=lambda c: cap[c])
                take = cap[c]
            core_pieces[c].append((take, t))
            cap[c] -= take
            wins[c] += -(-take // WIN)
            sz -= take
    assert all(x == 0 for x in cap)
    return core_pieces


def _pack_rows(tid):
    """Assign rows to cores (512 each) as contiguous single-task pieces;
    each piece becomes one 64-column head window (pieces > 64 rows get
    ceil(size/64) windows).

    Returns (order, core_windows, nwin): order[i] = original row index at
    packed position i (rows laid out core-major); core_windows[c] = list of
    (task, row_start, row_end, window_offset); nwin = shared window count.
    """
    counts = np.bincount(tid, minlength=NUM_TASKS)
    row_q = {t: list(np.nonzero(tid == t)[0]) for t in range(NUM_TASKS)
             if counts[t] > 0}

    def windows_of(core_pieces):
        core_windows = []
        for pieces in core_pieces:
            wins = []
            s = 0
            for sz, t in pieces:
                e = s + sz
                # cover piece [s, e) with 64-wide windows at even offsets
                # (offset = row start rounded down to even, clamped)
                ws = s
                while True:
                    off = min(ws & ~1, BPC - WIN)
                    we = min(e, off + WIN)
                    wins.append((t, ws, we, off))
                    if we >= e:
                        break
                    ws = we
                s = e
            assert s == BPC
            core_windows.append(wins)
        return core_windows

    cands = []
    try:
        cands.append(windows_of(_pack_dp(counts)))
    except AssertionError:
        pass
    cands.append(windows_of(_pack_seq(counts)))
    cands.append(windows_of(_pack_lpt(counts)))
    core_windows = min(cands, key=lambda cw: max(len(w) for w in cw))
    nwin = max(len(w) for w in core_windows)

    order = []
    for c in range(NCORES):
        pos = 0
        for t, ws, we, off in core_windows[c]:
            assert ws == pos and pos >= off and we <= off + WIN
            q = row_q[t]
            order.extend(q[:we - ws])
            row_q[t] = q[we - ws:]
            pos = we
        assert pos == BPC
    assert all(not q for q in row_q.values())
    return np.asarray(order), core_windows, nwin


def prepare(x, W0, b0, W1, b1, W2, b2, head_W, head_b):
    """Host-side sharding. Returns (in_maps, order, sorted_task_ids, nwin)."""
    x = np.asarray(x, np.float32)
    tid = np.argmax(x[:, -NUM_TASKS:], axis=1)
    order, core_windows, nwin = _pack_rows(tid)
    x_s = x[order]
    t_s = tid[order]

    fp = _fingerprint(W0, W1, W2, b0, b1, b2, head_W)
    cached = _PACK_CACHE.get("w")
    if cached is not None and cached[0] == fp:
        w0p, w1p, w2p, bias, W0oh, hw_pack = cached[1]
    else:
        W0 = np.asarray(W0, np.float32)
        w0p, w1p, w2p, bias = _pack_trunk(
            W0, np.asarray(W1, np.float32), np.asarray(W2, np.float32),
            np.asarray(b1, np.float32), np.asarray(b2, np.float32))
        # one-hot contribution rows: relu(x @ W0 + b0) = relu(feats @
        # W0[:2048] + W0[2048 + tid] + b0) -- last two terms host-gathered
        W0oh = W0[FEAT:FEAT + NUM_TASKS] + np.asarray(b0, np.float32)[None, :]
        head_W = np.asarray(head_W, np.float32).astype(NPBF16)
        # hw_pack[t, kp, kc*256 + j] = head_W[t, kc*128 + kp, j]
        hw_pack = np.ascontiguousarray(
            head_W.reshape(NUM_TASKS, KC, 128, HEAD_DIM)
            .transpose(0, 2, 1, 3)
            .reshape(NUM_TASKS, 128, KC * HEAD_DIM)
        )
        _PACK_CACHE["w"] = (fp, (w0p, w1p, w2p, bias, W0oh, hw_pack))

    in_maps = []
    for c in range(NCORES):
        xs = x_s[c * BPC:(c + 1) * BPC]
        xTp = np.ascontiguousarray(xs[:, :FEAT].T).astype(NPBF16)
        ts_c = t_s[c * BPC:(c + 1) * BPC]
        ct_c = np.ascontiguousarray(W0oh[ts_c].T).astype(NPBF16)
        offs_c = np.zeros((1, nwin), np.int32)
        msk_c = np.zeros((128, nwin * WIN), np.uint8)
        wtasks = []
        for w, (t, ws, we, off) in enumerate(core_windows[c]):
            offs_c[0, w] = off
            msk_c[:, w * WIN + (ws - off): w * WIN + (we - off)] = 1
            wtasks.append(t)
        hws_c = np.zeros((nwin, 128, KC * HEAD_DIM), NPBF16)
        hws_c[:len(wtasks)] = hw_pack[np.asarray(wtasks, np.int64)]
        in_maps.append({
            "xT": np.ascontiguousarray(
                xTp.reshape(KC // 2, 2, 128, BPC).transpose(0, 2, 1, 3)
            ).reshape(KC // 2, 128, 2 * BPC),
            "ctb": np.ascontiguousarray(
                ct_c.reshape(KC // 2, 2, 128, BPC).transpose(0, 2, 1, 3)
            ).reshape(KC // 2, 128, 2 * BPC),
            "w0p": w0p, "w1p": w1p, "w2p": w2p, "bias": bias,
            "hws": hws_c, "msk": msk_c, "offs": offs_c,
        })
    return in_maps, order, t_s, nwin


def _assemble(results, order, t_s, head_b):
    head_b = np.asarray(head_b, np.float32)
    outs = []
    for c in range(NCORES):
        o = results[c]["outT"]                       # [2, 128, BPC]
        outs.append(o.transpose(2, 0, 1).reshape(BPC, HEAD_DIM))
    out_s = np.concatenate(outs, axis=0) + head_b[t_s]
    out = np.empty_like(out_s)
    out[order] = out_s
    return out.astype(np.float32)


def kernel(x, W0, b0, W1, b1, W2, b2, head_W, head_b):
    in_maps, order, t_s, nwin = prepare(x, W0, b0, W1, b1, W2, b2, head_W, head_b)
    nc = _PROG_CACHE.get(nwin)
    if nc is None:
        nc = _build(nwin)
        _PROG_CACHE[nwin] = nc
    res = bass_utils.run_bass_kernel_spmd(nc, in_maps, core_ids=list(range(NCORES)))
    return _assemble(res.results, order, t_s, head_b)
